# revision 38
# baseline (speedup 1.0000x reference)
"""MDCT kernel for Trainium2 (8 NeuronCores, batch-parallel, folded DCT-IV).

Math: for frame f (hop N=1024, frame len 2N, center-padded), output bin k:
    out[b, f, k] = sum_n xp[b, f*N + n] * window[n] * C[n, k]
    C[n, k] = sqrt(2/N) * cos(pi/N * (n + 0.5 + N/2) * (k + 0.5))

C has exact reflection symmetries C[N-1-j,k] = -C[j,k] (first hop) and
C[2N-1-j,k] = +C[N+j,k] (second hop), so each windowed hop block folds to
512 values and the contraction halves:
    out[f] = M1-part @ uA[f-1] + M2-part @ uB[f]
    uA[r][j] = w[j]   * (x2[r][j] - (w[N-1-j]/w[j])    * x2[r][N-1-j])
    uB[r][j] = w[N+j] * (x2[r][j] + (w[2N-1-j]/w[N+j]) * x2[r][N-1-j])
(x2 = x.reshape(1024, 1024) hop blocks; uA[-1] = uB[1024] = 0 from padding.)

The leading window factor is absorbed into the matmul matrix rows
(M1 = diag(w[j]) C[0:512], M2 = diag(w[N+j]) C[N:N+512], cached per window),
so the on-chip fold is ONE scalar_tensor_tensor per output:
    u' = (xr * ratio) add xf      (per-partition ratio scalar)
The host pre-transposes x2 (reversing the second-half rows), converts to
bf16 (halves the input DMA; rel err ~2.9e-3 vs the 2e-2 gate), and lays
both inputs out with 2KB+ contiguous partition lines so the (16-queue
striped) DMA runs at full packet rate. Matmuls are bf16, PSUM f32.
"""

import numpy as np
import ml_dtypes

import concourse.bass as bass
import concourse.bacc as bacc
import concourse.mybir as mybir
import concourse.tile as tile
from concourse.bass_utils import run_bass_kernel_spmd

B = 8
T = 1 << 20
R = 1024          # hop blocks per channel (T // hop)
CN = 1024         # hop (= N)
NF = 1025         # output frames
NK = 1024         # output bins
F32 = mybir.dt.float32
BF16 = mybir.dt.bfloat16

_NC_CACHE = None
_MT_CACHE = None


def build_nc() -> bass.Bass:
    nc = bacc.Bacc("TRN2", target_bir_lowering=False, debug=False)
    # xt4[p, q, i, rl]: hop-transposed x, bf16, block-major:
    #   row j = 128*i + p of xt (j<512: x2[r, j]; j>=512: x2[r, 1535-j]),
    #   column r = 128*q + rl.  2KB contiguous per (p, q).
    xt = nc.dram_tensor("xt", [128, 8, 8, 128], BF16, kind="ExternalInput").ap()
    # mt[p, i, k] = M[128*i + p, k]; chunks 0..3 = diag(wa1) C[0:512],
    # chunks 4..7 = diag(wb1) C[1024:1536].  bf16, p-major.
    mt = nc.dram_tensor("mt", [128, 8, NK], BF16, kind="ExternalInput").ap()
    # wc[p, i]  i=0..3: -wa2/wa1 chunks,  i=4..7: wb2/wb1 chunks
    wc = nc.dram_tensor("wc", [128, 8], F32, kind="ExternalInput").ap()
    out = nc.dram_tensor("out", [NF, NK], F32, kind="ExternalOutput").ap()

    mult = mybir.AluOpType.mult
    add = mybir.AluOpType.add

    with tile.TileContext(nc) as tc:
        with (
            tc.tile_pool(name="persist", bufs=1) as persist,
            tc.tile_pool(name="xin", bufs=4) as xin,
            tc.tile_pool(name="outp", bufs=4) as outp,
            tc.tile_pool(name="mmps", bufs=6, space="PSUM") as mmps,
            tc.tile_pool(name="warmp", bufs=1, space="PSUM") as warmp,
        ):
            wct = persist.tile([128, 8], F32)
            mtile = persist.tile([128, 8, NK], BF16)

            # The PE sits idle from the end of the boot preamble (~6us)
            # until the first fold lands (~12us), and starts cold at the
            # 1.2GHz p-state (2.4GHz needs ~3us of continuous work).
            # Dummy matmuls bridge that window so the clock is already
            # ramped when real data arrives.
            warm = persist.tile([128, 512], BF16)
            nc.vector.memset(warm[:], 0.0)
            wps = warmp.tile([128, 512], F32)
            for _ in range(12):
                nc.tensor.matmul(wps[:], warm[:, 0:128], warm[:],
                                 start=True, stop=True)

            # u[p, ci, f]: ci<4 -> uA chunk at col f = uA[f-1] (col 0 zero);
            #              ci>=4 -> uB chunk at col f = uB[f] (col 1024 zero).
            u = persist.tile([128, 8, NF], BF16)

            # DMA issue order is the latency-critical path, and DMA queues
            # drain in issue order.  The first matmul tile runs its pa
            # (k-half 0) accumulation first, so only half of mt gates the
            # PE start; xt blocks 0-2 are woven between the mt halves, and
            # blocks 3-7 rotate through a 4-buffer pool that throttles them
            # behind the folds (keeping early HBM bandwidth on mt).
            xts = []

            def load_xt(q: int):
                xtq = xin.tile([128, 8, 128], BF16)
                nc.sync.dma_start(xtq[:], xt[:, q])
                xts.append(xtq)

            nc.sync.dma_start(wct[:], wc[:, :])
            load_xt(0)
            nc.sync.dma_start(mtile[:, 0:4, 0:512], mt[:, 0:4, 0:512])
            load_xt(1)
            nc.sync.dma_start(mtile[:, 4:8, 0:512], mt[:, 4:8, 0:512])
            nc.sync.dma_start(mtile[:, 0:4, 512:1024], mt[:, 0:4, 512:1024])
            load_xt(2)
            nc.sync.dma_start(mtile[:, 4:8, 512:1024], mt[:, 4:8, 512:1024])
            for q in range(3, 8):
                load_xt(q)

            nc.vector.memset(u[:, 0:4, 0:1], 0.0)
            nc.vector.memset(u[:, 4:8, NF - 1:NF], 0.0)

            def fold_block(q: int):
                c0 = q * 128
                xtq = xts[q]
                for i in range(4):
                    xf = xtq[:, i, :]
                    xr = xtq[:, 4 + i, :]
                    nc.vector.scalar_tensor_tensor(
                        u[:, i, c0 + 1:c0 + 129], xr, wct[:, i:i + 1], xf,
                        op0=mult, op1=add)
                    nc.vector.scalar_tensor_tensor(
                        u[:, 4 + i, c0:c0 + 128], xr, wct[:, 4 + i:5 + i], xf,
                        op0=mult, op1=add)

            def mm_half(f0: int, k0: int):
                ph = mmps.tile([128, 512], F32, tag="mm")
                ot = outp.tile([128, 512], F32)
                for ci in range(8):
                    nc.tensor.matmul(ph[:], u[:, ci, f0:f0 + 128],
                                     mtile[:, ci, k0:k0 + 512],
                                     start=(ci == 0), stop=(ci == 7))
                nc.scalar.copy(ot[:], ph[:])
                nc.gpsimd.dma_start(out[f0:f0 + 128, k0:k0 + 512], ot[:])

            def mm_tile(f0: int):
                mm_half(f0, 0)
                mm_half(f0, 512)

            def mm_pair(f0a: int, f0b: int, k0: int):
                # Two tiles' k-half accumulations with chunk-quads
                # interleaved, so each 512KB mt piece (chunks 0-3 / 4-7 of
                # one k-half) enables ~1.7us of PE work — matched to its
                # ~1.7us arrival time on the striped DMA.
                p0 = mmps.tile([128, 512], F32, tag="mm")
                p1 = mmps.tile([128, 512], F32, tag="mm")
                for ci in range(4):
                    nc.tensor.matmul(p0[:], u[:, ci, f0a:f0a + 128],
                                     mtile[:, ci, k0:k0 + 512],
                                     start=(ci == 0), stop=False)
                for ci in range(4):
                    nc.tensor.matmul(p1[:], u[:, ci, f0b:f0b + 128],
                                     mtile[:, ci, k0:k0 + 512],
                                     start=(ci == 0), stop=False)
                for ci in range(4, 8):
                    nc.tensor.matmul(p0[:], u[:, ci, f0a:f0a + 128],
                                     mtile[:, ci, k0:k0 + 512],
                                     start=False, stop=(ci == 7))
                for ci in range(4, 8):
                    nc.tensor.matmul(p1[:], u[:, ci, f0b:f0b + 128],
                                     mtile[:, ci, k0:k0 + 512],
                                     start=False, stop=(ci == 7))
                for f0, ph in ((f0a, p0), (f0b, p1)):
                    ot = outp.tile([128, 512], F32)
                    nc.scalar.copy(ot[:], ph[:])
                    nc.gpsimd.dma_start(out[f0:f0 + 128, k0:k0 + 512], ot[:])

            # Tiles 0 and 1 run both pa (k-half 0) accumulations before
            # either pb: the first mt megabyte then covers ~3.4us of PE
            # work, matching its arrival rate so the PE never starves
            # while the rest of mt streams in.
            fold_block(0)
            fold_block(1)
            mm_pair(0, 128, 0)
            fold_block(2)
            mm_pair(0, 128, 512)
            for t in range(2, 7):
                fold_block(t + 1)
                mm_tile(128 * t)

            mm_tile(896)

            # Last frame (f = 1024): only the A part (uA[1023], u col 1024).
            pa = mmps.tile([1, 512], F32, tag="mm")
            pb = mmps.tile([1, 512], F32, tag="mm")
            for ci in range(4):
                w_ = u[:, ci, 1024:1025]
                nc.tensor.matmul(pa[:], w_, mtile[:, ci, 0:512],
                                 start=(ci == 0), stop=(ci == 3))
                nc.tensor.matmul(pb[:], w_, mtile[:, ci, 512:1024],
                                 start=(ci == 0), stop=(ci == 3))
            ot = outp.tile([1, NK], F32, tag="ot_last")
            nc.scalar.copy(ot[:, 0:512], pa[:])
            nc.scalar.copy(ot[:, 512:1024], pb[:])
            nc.gpsimd.dma_start(out[1024:1025, :], ot[:])

    return nc


def _to_bf16(a: np.ndarray) -> np.ndarray:
    return a.astype(np.float32).astype(ml_dtypes.bfloat16)


def make_mt_wc(window: np.ndarray):
    w = window.astype(np.float64)
    n = np.arange(2 * NK, dtype=np.float64)[:, None]
    k = np.arange(NK, dtype=np.float64)[None, :]
    c = np.sqrt(2.0 / NK) * np.cos(np.pi / NK * (n + 0.5 + NK / 2) * (k + 0.5))
    j = np.arange(512)
    wa1, wa2 = w[j], w[1023 - j]
    wb1, wb2 = w[1024 + j], w[2047 - j]
    m = np.concatenate([wa1[:, None] * c[:512], wb1[:, None] * c[1024:1536]])
    mt = np.ascontiguousarray(
        _to_bf16(m).reshape(8, 128, NK).transpose(1, 0, 2))   # [128, 8, 1024]
    ratios = np.concatenate([-(wa2 / wa1), wb2 / wb1]).astype(np.float32)
    wc = np.ascontiguousarray(ratios.reshape(8, 128).T)       # [128, 8]
    return mt, wc


def make_xt(xb: np.ndarray) -> np.ndarray:
    x2t = np.ascontiguousarray(xb.reshape(R, CN).T)   # [j, r] f32
    x2t[512:] = x2t[512:][::-1]
    xt_bf = _to_bf16(x2t)                             # [1024, 1024] bf16
    # [j, r] -> [p, q, i, rl]: j = 128*i + p, r = 128*q + rl
    x4 = xt_bf.reshape(8, 128, 8, 128).transpose(1, 2, 0, 3)
    return np.ascontiguousarray(x4)                   # [128, 8, 8, 128]


def _get_nc() -> bass.Bass:
    global _NC_CACHE
    if _NC_CACHE is None:
        _NC_CACHE = build_nc()
        _NC_CACHE.compile()
    return _NC_CACHE


def run_spmd(x: np.ndarray, window: np.ndarray, **kwargs):
    """Shard, run on 8 cores, return (stacked output, BassKernelResults)."""
    global _MT_CACHE
    if _MT_CACHE is None or _MT_CACHE[0] != window.tobytes():
        _MT_CACHE = (window.tobytes(),) + make_mt_wc(window)
    mt, wc = _MT_CACHE[1], _MT_CACHE[2]
    in_maps = [
        {"xt": make_xt(x[b]), "mt": mt, "wc": wc} for b in range(B)
    ]
    res = run_bass_kernel_spmd(nc=_get_nc(), in_maps=in_maps,
                               core_ids=list(range(B)), **kwargs)
    out = np.stack([res.results[b]["out"] for b in range(B)], axis=0)
    return out, res


def kernel(x: np.ndarray, window: np.ndarray) -> np.ndarray:
    out, _ = run_spmd(np.asarray(x), np.asarray(window))
    return out


# revision 39
# speedup vs baseline: 1.0083x; 1.0083x over previous
"""MDCT kernel for Trainium2 (8 NeuronCores, batch-parallel, folded DCT-IV).

Math: for frame f (hop N=1024, frame len 2N, center-padded), output bin k:
    out[b, f, k] = sum_n xp[b, f*N + n] * window[n] * C[n, k]
    C[n, k] = sqrt(2/N) * cos(pi/N * (n + 0.5 + N/2) * (k + 0.5))

C has exact reflection symmetries C[N-1-j,k] = -C[j,k] (first hop) and
C[2N-1-j,k] = +C[N+j,k] (second hop), so each windowed hop block folds to
512 values and the contraction halves:
    out[f] = M1-part @ uA[f-1] + M2-part @ uB[f]
    uA[r][j] = w[j]   * (x2[r][j] - (w[N-1-j]/w[j])    * x2[r][N-1-j])
    uB[r][j] = w[N+j] * (x2[r][j] + (w[2N-1-j]/w[N+j]) * x2[r][N-1-j])
(x2 = x.reshape(1024, 1024) hop blocks; uA[-1] = uB[1024] = 0 from padding.)

The leading window factor is absorbed into the matmul matrix rows
(M1 = diag(w[j]) C[0:512], M2 = diag(w[N+j]) C[N:N+512], cached per window),
so the on-chip fold is ONE scalar_tensor_tensor per output:
    u' = (xr * ratio) add xf      (per-partition ratio scalar)
The host pre-transposes x2 (reversing the second-half rows), converts to
bf16 (halves the input DMA; rel err ~2.9e-3 vs the 2e-2 gate), and lays
both inputs out with 2KB+ contiguous partition lines so the (16-queue
striped) DMA runs at full packet rate. Matmuls are bf16, PSUM f32.
"""

import numpy as np
import ml_dtypes

import concourse.bass as bass
import concourse.bacc as bacc
import concourse.mybir as mybir
import concourse.tile as tile
from concourse.bass_utils import run_bass_kernel_spmd

B = 8
T = 1 << 20
R = 1024          # hop blocks per channel (T // hop)
CN = 1024         # hop (= N)
NF = 1025         # output frames
NK = 1024         # output bins
F32 = mybir.dt.float32
BF16 = mybir.dt.bfloat16

_NC_CACHE = None
_MT_CACHE = None


def build_nc() -> bass.Bass:
    nc = bacc.Bacc("TRN2", target_bir_lowering=False, debug=False)
    # xt4[p, q, i, rl]: hop-transposed x, bf16, block-major:
    #   row j = 128*i + p of xt (j<512: x2[r, j]; j>=512: x2[r, 1535-j]),
    #   column r = 128*q + rl.  2KB contiguous per (p, q).
    xt = nc.dram_tensor("xt", [128, 8, 8, 128], BF16, kind="ExternalInput").ap()
    # mt[p, i, k] = M[128*i + p, k]; chunks 0..3 = diag(wa1) C[0:512],
    # chunks 4..7 = diag(wb1) C[1024:1536].  bf16, p-major.
    mt = nc.dram_tensor("mt", [128, 8, NK], BF16, kind="ExternalInput").ap()
    # wc[p, i]  i=0..3: -wa2/wa1 chunks,  i=4..7: wb2/wb1 chunks
    wc = nc.dram_tensor("wc", [128, 8], F32, kind="ExternalInput").ap()
    out = nc.dram_tensor("out", [NF, NK], F32, kind="ExternalOutput").ap()

    mult = mybir.AluOpType.mult
    add = mybir.AluOpType.add

    with tile.TileContext(nc) as tc:
        with (
            tc.tile_pool(name="persist", bufs=1) as persist,
            tc.tile_pool(name="xin", bufs=4) as xin,
            tc.tile_pool(name="outp", bufs=4) as outp,
            tc.tile_pool(name="mmps", bufs=6, space="PSUM") as mmps,
            tc.tile_pool(name="warmp", bufs=1, space="PSUM") as warmp,
        ):
            wct = persist.tile([128, 8], F32)
            mtile = persist.tile([128, 8, NK], BF16)

            # The PE sits idle from the end of the boot preamble (~6us)
            # until the first fold lands (~12us), and starts cold at the
            # 1.2GHz p-state (2.4GHz needs ~3us of continuous work).
            # Dummy matmuls bridge that window so the clock is already
            # ramped when real data arrives.
            warm = persist.tile([128, 512], BF16)
            nc.vector.memset(warm[:], 0.0)
            wps = warmp.tile([128, 512], F32)
            for _ in range(12):
                nc.tensor.matmul(wps[:], warm[:, 0:128], warm[:],
                                 start=True, stop=True)

            # u[p, ci, f]: ci<4 -> uA chunk at col f = uA[f-1] (col 0 zero);
            #              ci>=4 -> uB chunk at col f = uB[f] (col 1024 zero).
            u = persist.tile([128, 8, NF], BF16)

            # DMA issue order is the latency-critical path, and DMA queues
            # drain in issue order.  The first matmul tile runs its pa
            # (k-half 0) accumulation first, so only half of mt gates the
            # PE start; xt blocks 0-2 are woven between the mt halves, and
            # blocks 3-7 rotate through a 4-buffer pool that throttles them
            # behind the folds (keeping early HBM bandwidth on mt).
            xts = []

            def load_xt(q: int):
                xtq = xin.tile([128, 8, 128], BF16)
                nc.sync.dma_start(xtq[:], xt[:, q])
                xts.append(xtq)

            nc.sync.dma_start(wct[:], wc[:, :])
            load_xt(0)
            nc.sync.dma_start(mtile[:, 0:4, 0:512], mt[:, 0:4, 0:512])
            load_xt(1)
            nc.sync.dma_start(mtile[:, 4:8, 0:512], mt[:, 4:8, 0:512])
            nc.sync.dma_start(mtile[:, 0:4, 512:1024], mt[:, 0:4, 512:1024])
            load_xt(2)
            nc.sync.dma_start(mtile[:, 4:8, 512:1024], mt[:, 4:8, 512:1024])
            for q in range(3, 8):
                load_xt(q)

            nc.vector.memset(u[:, 0:4, 0:1], 0.0)
            nc.vector.memset(u[:, 4:8, NF - 1:NF], 0.0)

            def fold_block(q: int):
                c0 = q * 128
                xtq = xts[q]
                for i in range(4):
                    xf = xtq[:, i, :]
                    xr = xtq[:, 4 + i, :]
                    nc.vector.scalar_tensor_tensor(
                        u[:, i, c0 + 1:c0 + 129], xr, wct[:, i:i + 1], xf,
                        op0=mult, op1=add)
                    nc.vector.scalar_tensor_tensor(
                        u[:, 4 + i, c0:c0 + 128], xr, wct[:, 4 + i:5 + i], xf,
                        op0=mult, op1=add)

            def mm_half(f0: int, k0: int):
                ph = mmps.tile([128, 512], F32, tag="mm")
                ot = outp.tile([128, 512], F32)
                for ci in range(8):
                    nc.tensor.matmul(ph[:], u[:, ci, f0:f0 + 128],
                                     mtile[:, ci, k0:k0 + 512],
                                     start=(ci == 0), stop=(ci == 7))
                nc.scalar.copy(ot[:], ph[:])
                nc.gpsimd.dma_start(out[f0:f0 + 128, k0:k0 + 512], ot[:])

            def mm_tile(f0: int):
                mm_half(f0, 0)
                mm_half(f0, 512)

            # Tiles 0 and 1 run both pa (k-half 0) accumulations before
            # either pb: the first mt megabyte then covers ~3.4us of PE
            # work, matching its arrival rate so the PE never starves
            # while the rest of mt streams in.
            fold_block(0)
            fold_block(1)
            mm_half(0, 0)
            mm_half(128, 0)
            fold_block(2)
            mm_half(0, 512)
            mm_half(128, 512)
            for t in range(2, 7):
                fold_block(t + 1)
                mm_tile(128 * t)

            mm_tile(896)

            # Last frame (f = 1024): only the A part (uA[1023], u col 1024).
            pa = mmps.tile([1, 512], F32, tag="mm")
            pb = mmps.tile([1, 512], F32, tag="mm")
            for ci in range(4):
                w_ = u[:, ci, 1024:1025]
                nc.tensor.matmul(pa[:], w_, mtile[:, ci, 0:512],
                                 start=(ci == 0), stop=(ci == 3))
                nc.tensor.matmul(pb[:], w_, mtile[:, ci, 512:1024],
                                 start=(ci == 0), stop=(ci == 3))
            ot = outp.tile([1, NK], F32, tag="ot_last")
            nc.scalar.copy(ot[:, 0:512], pa[:])
            nc.scalar.copy(ot[:, 512:1024], pb[:])
            nc.gpsimd.dma_start(out[1024:1025, :], ot[:])

    return nc


def _to_bf16(a: np.ndarray) -> np.ndarray:
    return a.astype(np.float32).astype(ml_dtypes.bfloat16)


def make_mt_wc(window: np.ndarray):
    w = window.astype(np.float64)
    n = np.arange(2 * NK, dtype=np.float64)[:, None]
    k = np.arange(NK, dtype=np.float64)[None, :]
    c = np.sqrt(2.0 / NK) * np.cos(np.pi / NK * (n + 0.5 + NK / 2) * (k + 0.5))
    j = np.arange(512)
    wa1, wa2 = w[j], w[1023 - j]
    wb1, wb2 = w[1024 + j], w[2047 - j]
    m = np.concatenate([wa1[:, None] * c[:512], wb1[:, None] * c[1024:1536]])
    mt = np.ascontiguousarray(
        _to_bf16(m).reshape(8, 128, NK).transpose(1, 0, 2))   # [128, 8, 1024]
    ratios = np.concatenate([-(wa2 / wa1), wb2 / wb1]).astype(np.float32)
    wc = np.ascontiguousarray(ratios.reshape(8, 128).T)       # [128, 8]
    return mt, wc


def make_xt(xb: np.ndarray) -> np.ndarray:
    x2t = np.ascontiguousarray(xb.reshape(R, CN).T)   # [j, r] f32
    x2t[512:] = x2t[512:][::-1]
    xt_bf = _to_bf16(x2t)                             # [1024, 1024] bf16
    # [j, r] -> [p, q, i, rl]: j = 128*i + p, r = 128*q + rl
    x4 = xt_bf.reshape(8, 128, 8, 128).transpose(1, 2, 0, 3)
    return np.ascontiguousarray(x4)                   # [128, 8, 8, 128]


def _get_nc() -> bass.Bass:
    global _NC_CACHE
    if _NC_CACHE is None:
        _NC_CACHE = build_nc()
        _NC_CACHE.compile()
    return _NC_CACHE


def run_spmd(x: np.ndarray, window: np.ndarray, **kwargs):
    """Shard, run on 8 cores, return (stacked output, BassKernelResults)."""
    global _MT_CACHE
    if _MT_CACHE is None or _MT_CACHE[0] != window.tobytes():
        _MT_CACHE = (window.tobytes(),) + make_mt_wc(window)
    mt, wc = _MT_CACHE[1], _MT_CACHE[2]
    in_maps = [
        {"xt": make_xt(x[b]), "mt": mt, "wc": wc} for b in range(B)
    ]
    res = run_bass_kernel_spmd(nc=_get_nc(), in_maps=in_maps,
                               core_ids=list(range(B)), **kwargs)
    out = np.stack([res.results[b]["out"] for b in range(B)], axis=0)
    return out, res


def kernel(x: np.ndarray, window: np.ndarray) -> np.ndarray:
    out, _ = run_spmd(np.asarray(x), np.asarray(window))
    return out
